# revision 21
# baseline (speedup 1.0000x reference)
"""BERT-CRF Viterbi decode kernel for Trainium2 (Bass/Tile), 8-core data parallel.

v2 design (alpha+beta formulation, host-pretransposed activations):

Full inputs in, full outputs out. Batch B=64 sharded across 8 cores (8 seqs
each). Per core, scan rows r = b*16 + c (c = chunk of L=32 timesteps).

  Host: sentences pre-transposed to [p=h%128, g, ch, uu, row] so the PE
  matmul rhs streams straight out of DMA (no on-device transposes), and
  W pre-transposed to [p, ch, k].

  Stage A (per group g of 4 steps): one 1.5MB DMA, 6 fp32r accumulating
  matmuls -> emissions^T [4,512] PSUM, tiny fix-transposes back to
  [rows,4], then (Vector) T'_u = trans+b+e_u and the forward max-plus
  recurrence Apre_u; (GpSimd) backward local suffix matrices Lv/F per
  group. All hidden under the ~35us DMA stream.

  Tail: forward boundary scan p2f (Vector) || suffix-group scan Ss + back
  boundary scan p2b (GpSimd); then alpha (p3f), beta (Ls o Ss o bb), and
  tags = first-argmax_j(alpha_u[j] + beta_u[j]) -- no backpointers, no
  backtracking.
"""
import sys
for p in ("/opt/trn_rl_repo", "/root/.axon_site/_ro/trn_rl_repo"):
    if p not in sys.path:
        sys.path.append(p)

import numpy as np
import concourse.bass as bass
import concourse.tile as tile
from concourse import mybir
from concourse.bass_utils import run_bass_kernel_spmd

F32 = mybir.dt.float32
F32R = mybir.dt.float32r
BF16 = mybir.dt.bfloat16
I32 = mybir.dt.int32
AX = mybir.AxisListType
OP = mybir.AluOpType

B, T, H, K = 64, 512, 768, 4
NCORES = 8
BC = B // NCORES          # 8 sequences per core
C, L = 16, 32             # chunks per sequence, steps per chunk
ROWS = BC * C             # 128 partition rows
HCH = H // 128            # 6 h-chunks
UG = 4                    # steps per group
NG = L // UG              # 8 groups
GW = HCH * UG * 128       # 3072 cols per partition per half-group
GW2 = 2 * GW              # hi | lo halves, bf16

NEG = -1.0e30

_NC_CACHE = {}


def build_nc():
    nc = bass.Bass()
    sentd = nc.declare_dram_parameter("sentT", [128, NG * GW2], BF16, isOutput=False)
    wtd = nc.declare_dram_parameter("wt", [128, HCH * 16], BF16, isOutput=False)
    # rowconsts[128, 64]: wfirst | iw | mpid | end | tbT | tbinitT | id4
    rcd = nc.declare_dram_parameter("rowconsts", [128, 64], F32, isOutput=False)
    tagsd = nc.declare_dram_parameter("tags", [BC, T], I32, isOutput=True)

    with tile.TileContext(nc) as tc:
        with tc.tile_pool(name="singles", bufs=1) as singles, \
             tc.tile_pool(name="gpool", bufs=NG) as gpool, \
             tc.tile_pool(name="et_pool", bufs=2) as et_pool, \
             tc.tile_pool(name="tmp_pool", bufs=2) as tmp_pool, \
             tc.tile_pool(name="gtmp_pool", bufs=2) as gtmp_pool, \
             tc.tile_pool(name="ps_eT", bufs=2, space="PSUM") as ps_eT, \
             tc.tile_pool(name="ps_fix", bufs=2, space="PSUM") as ps_fix:

            # ---------- constants (small, land first) ----------
            wt = singles.tile([128, HCH * 16], BF16)
            nc.sync.dma_start(wt, wtd[:])
            rc = singles.tile([128, 64], F32)
            nc.sync.dma_start(rc, rcd[:])
            wfirst = rc[:, 0:4]
            iw4 = rc[:, 4:8]
            mpid = rc[:, 8:24]          # max-plus identity (0 diag, NEG off)
            end_sb = rc[:, 24:28]
            tbT = rc[:, 28:44]          # tbT[j*4+k] = trans[k,j] + b[j]
            tbinitT = rc[:, 44:60]      # same with tinit (start row c==0)
            id84 = rc[:, 60:64]         # stacked [I4; I4] on partitions 0..7

            # ---------- prefetch all sentence groups ----------
            # Group 0 is split into sub-DMAs so the first matmul starts as
            # soon as its first h-chunks land instead of after the full 1.5MB.
            # Triggers are spread across the two HWDGE rings (Sync=SP and
            # Scalar=Act) -- each trigger costs ~650ns of issue time serially
            # per ring.
            gtiles = []
            for g in range(NG):
                gt = gpool.tile([128, GW2], BF16, tag="gt")
                if g == 0:
                    for c0, c1 in ((0, 1024), (1024, 3072), (3072, GW2)):
                        src = bass.AP(tensor=sentd[:].tensor, offset=c0,
                                      ap=[[NG * GW2, 128], [1, c1 - c0]])
                        nc.sync.dma_start(gt[:, c0:c1], src)
                elif g == NG - 1:
                    # split the last group too: its tail gates stage-A end
                    for c0, c1 in ((0, GW), (GW, GW2)):
                        src = bass.AP(tensor=sentd[:].tensor,
                                      offset=g * GW2 + c0,
                                      ap=[[NG * GW2, 128], [1, c1 - c0]])
                        nc.scalar.dma_start(gt[:, c0:c1], src)
                else:
                    # alternate HWDGE rings so consecutive groups stream in
                    # parallel (pairwise arrival matches PE consumption)
                    src = bass.AP(tensor=sentd[:].tensor, offset=g * GW2,
                                  ap=[[NG * GW2, 128], [1, GW2]])
                    eng = nc.sync if g % 2 == 0 else nc.scalar
                    eng.dma_start(gt, src)
                gtiles.append(gt)

            # ---------- persistent state ----------
            emsc = singles.tile([128, L * K], F32)
            emv = emsc.rearrange("p (u j) -> p u j", u=L)
            # T'^T storage: TT[u][j][k] = trans[k,j] + b[j] + e_u[j]
            TT = singles.tile([128, L, 4, 4], F32)
            # forward prefix mats Apre[u][i][j]
            Apre = singles.tile([128, L, 4, 4], F32)
            # backward local suffixes: LT[g][v][i][x] = Lv_{g,v}[i][x]
            LT = singles.tile([128, NG, UG, 4, 4], F32)
            # full-group transfers: FT[g][i][x] = F_g[i][x]
            FT = singles.tile([128, NG, 4, 4], F32)
            # suffix-of-groups, transposed: SsT[g][z][a] = Ss_g[a][z]
            SsT = singles.tile([128, NG, 4, 4], F32)

            # LT[g][3] = max-plus identity, all groups at once
            nc.vector.tensor_copy(
                LT[:, :, 3, :, :],
                mpid.rearrange("p (x i) -> p x i", x=4)
                    .unsqueeze(1).to_broadcast((128, NG, 4, 4)))

            # ---------- Stage A ----------
            sA = nc.named_scope("stageA")
            sA.__enter__()
            for g in range(NG):
                gt = gtiles[g]
                ghi = gt[:, 0:GW].rearrange("p (ch n) -> p ch n", ch=HCH)
                glo = gt[:, GW:GW2].rearrange("p (ch n) -> p ch n", ch=HCH)
                # eT8 rows 0-3: Shi*Whi + Slo*Whi; rows 4-7: Shi*Wlo
                eT_ps = ps_eT.tile([8, UG * 128], F32, tag="eT")
                for ch in range(HCH):
                    nc.tensor.matmul(
                        eT_ps,
                        wt[:, ch * 16:ch * 16 + 8],      # [Whi | Wlo]
                        ghi[:, ch, :],
                        start=(ch == 0), stop=False)
                for ch in range(HCH):
                    nc.tensor.matmul(
                        eT_ps,
                        wt[:, ch * 16 + 8:ch * 16 + 16],  # [Whi | 0]
                        glo[:, ch, :],
                        start=False, stop=(ch == HCH - 1))
                eT_sb = et_pool.tile([8, UG * 128], F32, tag="eTsb")
                nc.scalar.copy(eT_sb, eT_ps)
                for uu in range(UG):
                    u = g * UG + uu
                    fx = ps_fix.tile([128, K], F32, tag="fix")
                    # true matmul with rhs=[I4;I4] transposes AND sums the
                    # two row-quads: fx[row, k] = eT8[k, col] + eT8[4+k, col]
                    nc.tensor.matmul(
                        fx, eT_sb[:, uu * 128:(uu + 1) * 128], id84[0:8, :],
                        start=True, stop=True)
                    nc.scalar.copy(emsc[:, u * 4:(u + 1) * 4], fx)
                # ---- T'^T for this group: TT[u][j][k] = e_u[j] + tbT[j,k]
                if g == 0:
                    nc.vector.tensor_tensor(
                        TT[:, 0, :, :],
                        emv[:, 0, :].unsqueeze(2).to_broadcast((128, 4, 4)),
                        tbinitT.rearrange("p (j k) -> p j k", j=4),
                        OP.add)
                    nc.vector.tensor_tensor(
                        TT[:, 1:UG, :, :],
                        emv[:, 1:UG, :].unsqueeze(3).to_broadcast((128, UG - 1, 4, 4)),
                        tbT.rearrange("p (j k) -> p j k", j=4)
                           .unsqueeze(1).to_broadcast((128, UG - 1, 4, 4)),
                        OP.add)
                else:
                    nc.vector.tensor_tensor(
                        TT[:, g * UG:(g + 1) * UG, :, :],
                        emv[:, g * UG:(g + 1) * UG, :]
                            .unsqueeze(3).to_broadcast((128, UG, 4, 4)),
                        tbT.rearrange("p (j k) -> p j k", j=4)
                           .unsqueeze(1).to_broadcast((128, UG, 4, 4)),
                        OP.add)
                # ---- forward recurrence (Vector)
                for uu in range(UG):
                    u = g * UG + uu
                    if u == 0:
                        nc.scalar.copy(
                            Apre[:, 0, :, :], TT[:, 0, :, :].transpose([0, 2, 1]))
                    else:
                        t4 = tmp_pool.tile([128, 4, 4, 4], F32, tag="fwd")
                        # t4[i,j,k] = Apre[u-1][i,k] + TT[u][j,k]
                        nc.vector.tensor_tensor(
                            t4,
                            Apre[:, u - 1, :, :].unsqueeze(2).to_broadcast((128, 4, 4, 4)),
                            TT[:, u, :, :].unsqueeze(1).to_broadcast((128, 4, 4, 4)),
                            OP.add)
                        nc.vector.reduce_max(Apre[:, u, :, :], t4, axis=AX.X)
                # ---- backward local suffixes (Vector compose; Scalar copy)
                def bw_compose(out_ix, prev_ix, TTu_jk):
                    # out[i][x] = max_j T'[i][j] + prev[j][x]
                    tb4 = gtmp_pool.tile([128, 4, 4, 4], F32, tag="bwd")
                    nc.vector.tensor_tensor(
                        tb4,
                        prev_ix.transpose([0, 2, 1])      # [x, j]
                               .unsqueeze(1).to_broadcast((128, 4, 4, 4)),
                        TTu_jk.transpose([0, 2, 1])       # [i, j] (= T')
                              .unsqueeze(2).to_broadcast((128, 4, 4, 4)),
                        OP.add)                            # tb4[i, x, j]
                    nc.vector.reduce_max(out_ix, tb4, axis=AX.X)

                t3 = g * UG + 3
                nc.scalar.copy(
                    LT[:, g, 2, :, :], TT[:, t3, :, :].transpose([0, 2, 1]))
                for v in (1, 0):
                    bw_compose(LT[:, g, v, :, :], LT[:, g, v + 1, :, :],
                               TT[:, g * UG + v + 1, :, :])
                if g > 0:
                    bw_compose(FT[:, g, :, :], LT[:, g, 0, :, :],
                               TT[:, g * UG, :, :])
            sA.__exit__(None, None, None)

            # ---------- Ss: suffix-of-groups scan (Vector) ----------
            _sss = nc.named_scope("ssscan")
            _sss.__enter__()
            nc.vector.tensor_copy(
                SsT[:, NG - 1, :, :], mpid.rearrange("p (x i) -> p x i", x=4))
            for g in range(NG - 2, -1, -1):
                # SsT[g][z][a] = max_m SsT[g+1][z][m] + F_{g+1}[a][m]
                t4 = tmp_pool.tile([128, 4, 4, 4], F32, tag="fwd")
                nc.vector.tensor_tensor(
                    t4,
                    SsT[:, g + 1, :, :].unsqueeze(2).to_broadcast((128, 4, 4, 4)),
                    FT[:, g + 1, :, :]                 # [a, m]
                      .unsqueeze(1).to_broadcast((128, 4, 4, 4)),
                    OP.add)
                nc.vector.reduce_max(SsT[:, g, :, :], t4, axis=AX.X)
            _sss.__exit__(None, None, None)

            # ---------- regroup A_c to by-b layout ----------
            # Split so the high-c half (consumed first by p2b) lands first.
            _sp2 = nc.named_scope("p2")
            _sp2.__enter__()
            abyb = singles.tile([BC, C * 16], F32)
            nc.sync.dma_start(abyb, Apre[:, L - 1, :, :].rearrange("p a b -> p (a b)"))
            abv = abyb.rearrange("p (c i j) -> p c i j", c=C, i=4)

            # ----- p2b: backward boundary scores, rows 0..7 -----
            bby = singles.tile([BC, C * 4], F32)
            bbv = bby.rearrange("p (c j) -> p c j", c=C)
            nc.scalar.copy(bbv[:, C - 1, :], end_sb[0:BC, :])
            for c in range(C - 1, 0, -1):
                # bb_{c-1}[i] = max_j (A_c[i,j] + bb_c[j])
                p2tmp = gtmp_pool.tile([BC, 4, 4], F32, tag="p2b")
                nc.vector.tensor_tensor(
                    p2tmp,
                    abv[:, c, :, :],
                    bbv[:, c, :].unsqueeze(1).to_broadcast((BC, 4, 4)),
                    OP.add)
                nc.vector.reduce_max(bbv[:, c - 1, :], p2tmp, axis=AX.X)
            # broadcast bb to rows: bbc[128, 4], row b*16+c = bb_c[b]
            bbc = singles.tile([128, 4], F32)
            nc.sync.dma_start(bbc, bby)

            # ----- p2f: forward boundary scores, rows 0..7 -----
            sbound = singles.tile([BC, (C + 1) * 4], F32)
            nc.vector.memset(sbound[:, 0:4], 0.0)
            sbv = sbound.rearrange("p (c j) -> p c j", c=C + 1)
            for c in range(C):
                p2tmp = tmp_pool.tile([BC, 4, 4], F32, tag="p2f")
                # tmp[j,i] = s[i] + A_c[i,j]
                nc.vector.tensor_tensor(
                    p2tmp,
                    sbv[:, c, :].unsqueeze(1).to_broadcast((BC, 4, 4)),
                    abv[:, c, :, :].transpose([0, 2, 1]),
                    OP.add)
                nc.vector.reduce_max(sbv[:, c + 1, :], p2tmp, axis=AX.X)
            _sp2.__exit__(None, None, None)

            # ---------- p3f: alpha for all steps (Vector) ----------
            _sp3 = nc.named_scope("p3")
            _sp3.__enter__()
            scores = singles.tile([128, (L + 1) * 4], F32)
            nc.sync.dma_start(scores[:, 0:4], sbound[:, 0:C * 4])
            scv = scores.rearrange("p (u i) -> p u i", u=L + 1)
            p3tmp = singles.tile([128, L, 4, 4], F32)   # [u, j, i]
            nc.vector.tensor_tensor(
                p3tmp,
                scores[:, 0:4].unsqueeze(1).unsqueeze(1).to_broadcast((128, L, 4, 4)),
                Apre.transpose([0, 1, 3, 2]),
                OP.add)
            nc.vector.reduce_max(scv[:, 1:, :], p3tmp, axis=AX.X)

            # ---------- beta for all steps ----------
            # bsub[g][a] = max_z Ss_g[a][z] + bb_row[z]   (GpSimd)
            bsub = singles.tile([128, NG, 4], F32)
            bst = gtmp_pool.tile([128, NG, 4, 4], F32, tag="bst")
            nc.vector.tensor_tensor(
                bst,
                SsT.transpose([0, 1, 3, 2]),
                bbc.unsqueeze(1).unsqueeze(1).to_broadcast((128, NG, 4, 4)),
                OP.add)
            nc.vector.reduce_max(bsub, bst, axis=AX.X)
            # beta[u=(g,v)][i] = max_x Lv_{g,v}[i][x] + bsub[g][x]
            # LT viewed [p, g, (v i), x] keeps ops within 3 free dims.
            beta = singles.tile([128, L * 4], F32)
            bev = beta.rearrange("p (g vi) -> p g vi", g=NG)   # vi = v*4+i
            beuv = beta.rearrange("p (u i) -> p u i", u=L)
            LTm = LT.rearrange("p g v i x -> p g (v i) x")
            VI = UG * 4
            btmp = singles.tile([128, NG, VI, 4], F32)
            nc.vector.tensor_tensor(
                btmp,
                LTm,
                bsub.unsqueeze(2).to_broadcast((128, NG, VI, 4)),
                OP.add)
            nc.vector.reduce_max(bev, btmp, axis=AX.X)
            _sp3.__exit__(None, None, None)

            # ---------- tags: first-argmax_j(alpha+beta) ----------
            _sp5 = nc.named_scope("p5")
            _sp5.__enter__()
            delta = singles.tile([128, L, 4], F32)
            nc.vector.tensor_tensor(delta, scv[:, 1:, :], beuv, OP.add)
            mx = tmp_pool.tile([128, L], F32, tag="mx")
            nc.vector.reduce_max(mx, delta, axis=AX.X)
            eq = singles.tile([128, L, 4], F32)
            nc.vector.tensor_tensor(
                eq, delta, mx.unsqueeze(2).to_broadcast((128, L, 4)), OP.is_equal)
            nc.vector.tensor_tensor(
                eq, eq, wfirst.unsqueeze(1).to_broadcast((128, L, 4)), OP.mult)
            nc.vector.reduce_max(mx, eq, axis=AX.X)
            nc.vector.tensor_tensor(
                eq, eq, mx.unsqueeze(2).to_broadcast((128, L, 4)), OP.is_equal)
            nc.vector.tensor_tensor(
                eq, eq, iw4.unsqueeze(1).to_broadcast((128, L, 4)), OP.mult)
            tagf = tmp_pool.tile([128, L], F32, tag="tagf")
            nc.vector.reduce_sum(tagf, eq, axis=AX.X)
            tagi = tmp_pool.tile([128, L], I32, tag="tagi")
            nc.vector.tensor_copy(tagi, tagf)
            nc.sync.dma_start(tagsd[:].rearrange("b (c t) -> b c t", c=C), tagi)
            _sp5.__exit__(None, None, None)

    return nc


def _split_multi_waits(nc):
    """Walrus (bass2jax path) allows very few embedded sync waits per
    instruction (PE matmul: exactly 1). Hoist multi-waits onto standalone
    single-wait InstDrain instructions on the same engine, preserving order."""
    for f in nc.m.functions:
        for blk in f.blocks:
            insts = blk.instructions
            i = 0
            while i < len(insts):
                ins = insts[i]
                si = ins.sync_info
                w = list(si.on_wait) if (si is not None and si.on_wait) else []
                if len(w) >= 2:
                    for k, wait in enumerate(w):
                        d = mybir.InstEventSemaphore(
                            name=nc.get_next_instruction_name(), ins=[], outs=[])
                        d.engine = ins.engine
                        d.sync_info = mybir.SyncInfo(on_wait=[wait], on_update=[])
                        insts.insert(i + k, d)
                    i += len(w)
                    ins.sync_info = mybir.SyncInfo(
                        on_wait=[], on_update=list(si.on_update or []))
                i += 1


def _get_nc():
    if "nc" not in _NC_CACHE:
        nc = build_nc()
        _split_multi_waits(nc)   # HW path only; CoreSim rejects raw drains
        _NC_CACHE["nc"] = nc
    return _NC_CACHE["nc"]


def _bf16_round(x):
    """Round fp32 -> bf16 (RNE), returned as fp32 holding the bf16 value."""
    u = x.astype(np.float32).view(np.uint32)
    r = (u + 0x8000 + ((u >> 16) & 1)) & 0xFFFF0000
    return r.view(np.float32)


def _to_bf16(x):
    import ml_dtypes
    return x.astype(ml_dtypes.bfloat16)


def make_in_maps(inputs):
    sent = np.ascontiguousarray(np.asarray(inputs["sentences"], dtype=np.float32))
    W = np.ascontiguousarray(np.asarray(inputs["W"], dtype=np.float32))
    bb = np.asarray(inputs["b"], dtype=np.float32)
    st = np.asarray(inputs["start_transitions"], dtype=np.float32)
    en = np.asarray(inputs["end_transitions"], dtype=np.float32)
    tr = np.asarray(inputs["transitions"], dtype=np.float32)

    # W^T chunks, bf16 hi/lo: wt[p, ch*16 + (0:8)] = [Whi | Wlo],
    #                         wt[p, ch*16 + (8:16)] = [Whi | 0]
    wT = np.transpose(W.reshape(K, HCH, 128), (2, 1, 0))   # [p, ch, k]
    whi = _bf16_round(wT)
    wlo = _bf16_round(wT - whi)
    wt = np.zeros((128, HCH, 16), dtype=np.float32)
    wt[:, :, 0:4] = whi
    wt[:, :, 4:8] = wlo
    wt[:, :, 8:12] = whi
    wt = _to_bf16(np.ascontiguousarray(wt.reshape(128, HCH * 16)))

    # tbT[j*4+k] = trans[k,j] + b[j]
    tbT = (tr.T + bb[:, None]).reshape(16).astype(np.float32)  # [j,k] row-major
    # tinit rows: c==0 -> start (indep of k), else trans
    tinitT = np.tile(tbT, (128, 1))
    tbinit0 = (np.tile(st[:, None], (1, 4)) + bb[:, None]).reshape(16)
    tinitT[0::C, :] = tbinit0[None, :]

    mpid = (np.where(np.eye(4, dtype=bool), 0.0, NEG)).astype(np.float32).ravel()

    rc = np.zeros((128, 64), dtype=np.float32)
    rc[:, 0:4] = [4.0, 3.0, 2.0, 1.0]
    rc[:, 4:8] = [0.0, 1.0, 2.0, 3.0]
    rc[:, 8:24] = mpid[None, :]
    rc[:, 24:28] = en[None, :]
    rc[:, 28:44] = tbT[None, :]
    rc[:, 44:60] = tinitT
    rc[0:4, 60:64] = np.eye(4, dtype=np.float32)
    rc[4:8, 60:64] = np.eye(4, dtype=np.float32)

    in_maps = []
    for core in range(NCORES):
        sc = sent[core * BC:(core + 1) * BC]           # [8, 512, 768]
        s6 = sc.reshape(BC, C, NG, UG, HCH, 128)       # b c g uu ch p
        sT = np.transpose(s6, (5, 2, 4, 3, 0, 1))      # p g ch uu b c
        sT = np.ascontiguousarray(sT.reshape(128, NG, GW))
        shi = _bf16_round(sT)
        slo = _bf16_round(sT - shi)
        packed = np.concatenate([shi, slo], axis=2)    # [128, NG, 2*GW]
        packed = _to_bf16(np.ascontiguousarray(packed.reshape(128, NG * GW2)))
        in_maps.append({
            "sentT": packed, "wt": wt, "rowconsts": rc,
        })
    return in_maps


def kernel(**inputs):
    nc = _get_nc()
    in_maps = make_in_maps(inputs)
    res = run_bass_kernel_spmd(nc, in_maps, core_ids=list(range(NCORES)))
    tags = np.concatenate([res.results[c]["tags"] for c in range(NCORES)], axis=0)
    return tags.astype(np.int32)


if __name__ == "__main__":
    import reference
    inputs = {k: np.asarray(v) for k, v in reference.setup_inputs().items()}
    out = kernel(**inputs)
    print(out.shape, out.dtype, out[:2, :16])
